# revision 1
# baseline (speedup 1.0000x reference)
"""Distributed AttendedSeqEmbedding kernel for 8 trn2 NeuronCores.

Strategy (per sharding hint): data-parallel over batch B=512 across the 8
cores (64 sequences/core); GRU/MLP weights replicated. Each core runs the
full bidirectional masked-GRU scan + MLP attention pooling on its shard;
results are gathered to the full [B, 2H] output.

Hardcoded problem shapes: B=512, L=256, D=256, H=128, O=256.
"""

import numpy as np

B, L, D, H, O = 512, 256, 256, 128, 256
N_CORES = 8
BS = B // N_CORES  # 64 sequences per core


def _build_sharded_fn():
    import jax
    import jax.numpy as jnp

    def gru(xs, mask, Wih, Whh, bih, bhh, reverse):
        # xs: [L, bs, D], mask: [L, bs, 1] -> hidden states [L, bs, H]
        def step(h, inp):
            x_t, m_t = inp
            gx = x_t @ Wih.T + bih
            gh = h @ Whh.T + bhh
            xr, xz, xn = jnp.split(gx, 3, axis=-1)
            hr, hz, hn = jnp.split(gh, 3, axis=-1)
            r = jax.nn.sigmoid(xr + hr)
            z = jax.nn.sigmoid(xz + hz)
            cand = jnp.tanh(xn + r * hn)
            h_new = (1.0 - z) * cand + z * h
            h = jnp.where(m_t, h_new, h)
            return h, h

        h0 = jnp.zeros((xs.shape[1], Whh.shape[1]), xs.dtype)
        _, hs = jax.lax.scan(step, h0, (xs, mask), reverse=reverse)
        return hs

    def shard_fn(sequences, seqlens, Wih_f, Whh_f, bih_f, bhh_f,
                 Wih_b, Whh_b, bih_b, bhh_b, W_mlp, b_mlp, ctx):
        # sequences: [bs, L, D] local shard
        mask = jnp.arange(L)[None, :] < seqlens[:, None]
        xs = sequences.transpose(1, 0, 2)
        m = mask.T[:, :, None]
        hf = gru(xs, m, Wih_f, Whh_f, bih_f, bhh_f, False)
        hb = gru(xs, m, Wih_b, Whh_b, bih_b, bhh_b, True)
        rnn_out = jnp.concatenate([hf, hb], axis=-1).transpose(1, 0, 2)
        mlp_out = jnp.tanh(rnn_out @ W_mlp.T + b_mlp)
        scores = jnp.einsum('blo,o->bl', mlp_out, ctx)
        neg = jnp.finfo(scores.dtype).min
        w = jax.nn.softmax(jnp.where(mask, scores, neg), axis=1)
        return jnp.einsum('bl,blh->bh', w, rnn_out)

    # batch axes: sequences/seqlens sharded on axis 0, weights replicated
    in_axes = (0, 0) + (None,) * 11
    return jax.pmap(shard_fn, in_axes=in_axes, devices=jax.devices()[:N_CORES])


_PMAP_FN = None


def kernel(sequences, seqlens, Wih_f, Whh_f, bih_f, bhh_f,
           Wih_b, Whh_b, bih_b, bhh_b, W_mlp, b_mlp, ctx):
    global _PMAP_FN
    if _PMAP_FN is None:
        _PMAP_FN = _build_sharded_fn()

    seq_sh = np.ascontiguousarray(
        np.asarray(sequences, dtype=np.float32).reshape(N_CORES, BS, L, D))
    len_sh = np.ascontiguousarray(
        np.asarray(seqlens, dtype=np.int32).reshape(N_CORES, BS))

    out = _PMAP_FN(seq_sh, len_sh,
                   np.asarray(Wih_f, np.float32), np.asarray(Whh_f, np.float32),
                   np.asarray(bih_f, np.float32), np.asarray(bhh_f, np.float32),
                   np.asarray(Wih_b, np.float32), np.asarray(Whh_b, np.float32),
                   np.asarray(bih_b, np.float32), np.asarray(bhh_b, np.float32),
                   np.asarray(W_mlp, np.float32), np.asarray(b_mlp, np.float32),
                   np.asarray(ctx, np.float32))
    out = np.asarray(out)  # [8, 64, 2H]
    return out.reshape(B, 2 * H).astype(np.float32)
